# revision 1
# baseline (speedup 1.0000x reference)
"""Trainium2 Bass kernel: cache-distance -> exp kernel -> vocab histogram -> log_softmax.

Math (per cache row i): kern_i = exp(||cache_h[i] - h_t|| / 0.2)
                        cache_p[v] = sum_{i: word_ids[i]==v} kern_i
                        out = log_softmax(cache_p)[None, :]

Device strategy (8 cores, cache sharded along N):
  - cache slice uploaded pre-transposed [D=512, 32768] so D lives on SBUF partitions
  - ACT computes (x - h)^2 via Square activation with per-partition bias -h
  - PE reduces over D with one-hot-column [128,8] lhsT matmuls accumulating
    squared distances for 8 row-groups into one PSUM tile [8, 512]
  - ACT: kern = exp(exp(0.5*ln(25*d2)))  (Ln+Exp share one table set; avoids Sqrt)
  - PE transposes kern into [128, batch] orientation
  - histogram via outer-product matmul: hist[m,n] += sum_p kern_p *
      (m == wid_p % 128) * (n == wid_p // 128), with one-hots built by DVE
      fused tensor_scalar (is_equal, mult) against iota constants; PSUM
      accumulates all 256 batches into a single [128, 394] f32 tile.
Host: sum the 8 partial histograms, reorder to [V], log_softmax (tiny).
"""

import os
import sys

for _p in ("/root/.axon_site", "/root/.axon_site/_ro/trn_rl_repo",
           "/root/.axon_site/_ro/pypackages"):
    if os.path.isdir(_p) and _p not in sys.path:
        sys.path.append(_p)

import numpy as np

VOCAB = 50257
N_CACHE = 262144
D = 512
SMOOTH = 0.2
NCORES = 8
RPC = N_CACHE // NCORES        # 32768 rows per core
NCHUNK = 8
CHUNK = RPC // NCHUNK          # 4096 rows per chunk
GPC = CHUNK // 512             # 8 groups of 512 rows per chunk
NV = 394                       # hist free dim: wid // 128 in [0, 393), padded even
BPC = RPC // 128               # 256 batches of 128 elements per core

_CACHE = {}


def _build_program():
    import concourse.bacc as bacc
    import concourse.tile as tile
    import concourse.mybir as mybir

    f32, f16 = mybir.dt.float32, mybir.dt.float16
    AF = mybir.ActivationFunctionType
    ALU = mybir.AluOpType

    nc = bacc.Bacc("TRN2", target_bir_lowering=False, debug=False,
                   num_devices=NCORES)

    xt_d = nc.dram_tensor("xt", [D, RPC], f32, kind="ExternalInput")
    am_d = nc.dram_tensor("am", [128, BPC], f32, kind="ExternalInput")
    bn_d = nc.dram_tensor("bn", [128, BPC], f32, kind="ExternalInput")
    nh_d = nc.dram_tensor("nh", [128, 4], f32, kind="ExternalInput")
    im_d = nc.dram_tensor("im", [128, 128], f16, kind="ExternalInput")
    in_d = nc.dram_tensor("inn", [128, NV], f16, kind="ExternalInput")
    oh_d = nc.dram_tensor("oh", [128, 64], f16, kind="ExternalInput")
    id_d = nc.dram_tensor("idm", [8, 8], f32, kind="ExternalInput")
    hist_d = nc.dram_tensor("hist", [128, NV], f32, kind="ExternalOutput")

    with tile.TileContext(nc) as tc:
        with (
            tc.tile_pool(name="const", bufs=1) as cpool,
            tc.tile_pool(name="x", bufs=3) as xpool,
            tc.tile_pool(name="sq", bufs=3) as sqpool,
            tc.tile_pool(name="s", bufs=2) as spool,
            tc.tile_pool(name="kt", bufs=2) as ktpool,
            tc.tile_pool(name="a", bufs=4) as apool,
            tc.tile_pool(name="b", bufs=4) as bpool,
            tc.tile_pool(name="out", bufs=1) as opool,
            tc.tile_pool(name="pshist", bufs=1, space="PSUM") as pshist,
            tc.tile_pool(name="psdist", bufs=2, space="PSUM") as psdist,
            tc.tile_pool(name="pskt", bufs=2, space="PSUM") as pskt,
        ):
            am = cpool.tile([128, BPC], f32)
            nc.sync.dma_start(am[:], am_d.ap())
            bn = cpool.tile([128, BPC], f32)
            nc.sync.dma_start(bn[:], bn_d.ap())
            nh = cpool.tile([128, 4], f32)
            nc.sync.dma_start(nh[:], nh_d.ap())
            im = cpool.tile([128, 128], f16)
            nc.sync.dma_start(im[:], im_d.ap())
            inn = cpool.tile([128, NV], f16)
            nc.sync.dma_start(inn[:], in_d.ap())
            oh = cpool.tile([128, 64], f16)
            nc.sync.dma_start(oh[:], oh_d.ap())
            idm = cpool.tile([8, 8], f32)
            nc.sync.dma_start(idm[:], id_d.ap())

            hist = pshist.tile([128, NV], f32)
            xt_ap = xt_d.ap()

            for ch in range(NCHUNK):
                dist = psdist.tile([8, 512], f32)
                mm = 0
                for c in range(4):
                    x = xpool.tile([128, CHUNK], f32)
                    nc.sync.dma_start(
                        x[:],
                        xt_ap[c * 128:(c + 1) * 128,
                              ch * CHUNK:(ch + 1) * CHUNK],
                    )
                    sq = sqpool.tile([128, CHUNK], f16)
                    nc.scalar.activation(sq[:], x[:], AF.Square,
                                         bias=nh[:, c:c + 1])
                    for g in range(GPC):
                        nc.tensor.matmul(
                            dist[:],
                            oh[:, g * 8:(g + 1) * 8],
                            sq[:, g * 512:(g + 1) * 512],
                            start=(mm == 0),
                            stop=(mm == 4 * GPC - 1),
                        )
                        mm += 1
                lg = spool.tile([8, 512], f32)
                nc.scalar.activation(lg[:], dist[:], AF.Ln, scale=25.0)
                d5 = spool.tile([8, 512], f32)
                nc.scalar.activation(d5[:], lg[:], AF.Exp, scale=0.5)
                kern = spool.tile([8, 512], f32)
                nc.scalar.activation(kern[:], d5[:], AF.Exp)

                ktp = pskt.tile([128, 32], f32)
                for c4 in range(4):
                    nc.tensor.transpose(
                        ktp[:, c4 * 8:(c4 + 1) * 8],
                        kern[:, c4 * 128:(c4 + 1) * 128],
                        idm[:],
                    )
                kt = ktpool.tile([128, 32], f32)
                nc.scalar.copy(kt[:], ktp[:])

                for c4 in range(4):
                    for g in range(GPC):
                        bl = c4 * 8 + g
                        b = ch * 32 + bl
                        A = apool.tile([128, 128], f16)
                        nc.vector.tensor_scalar(
                            A[:], im[:], am[:, b:b + 1], kt[:, bl:bl + 1],
                            ALU.is_equal, ALU.mult,
                        )
                        B = bpool.tile([128, NV], f16)
                        nc.vector.tensor_scalar(
                            B[:], inn[:], bn[:, b:b + 1], None, ALU.is_equal,
                        )
                        nc.tensor.matmul(
                            hist[:], A[:], B[:],
                            start=(b == 0), stop=(b == BPC - 1),
                        )

            hist_sb = opool.tile([128, NV], f32)
            nc.scalar.copy(hist_sb[:], hist[:])
            nc.sync.dma_start(hist_d.ap(), hist_sb[:])

    nc.compile()
    return nc


def _prep_inputs(h_t, cache_h, word_ids):
    h_t = np.asarray(h_t, dtype=np.float32)
    cache_h = np.ascontiguousarray(np.asarray(cache_h, dtype=np.float32))
    word_ids = np.asarray(word_ids, dtype=np.int64)

    # [core, D, RPC] transposed cache slices
    xt8 = np.ascontiguousarray(
        cache_h.reshape(NCORES, RPC, D).transpose(0, 2, 1))

    w8 = word_ids.reshape(NCORES, NCHUNK, GPC, 4, 128)  # [core, ch, g, c, p]
    am8 = np.ascontiguousarray(
        (w8 % 128).astype(np.float32).transpose(0, 4, 1, 3, 2)
    ).reshape(NCORES, 128, BPC)
    bn8 = np.ascontiguousarray(
        (w8 // 128).astype(np.float32).transpose(0, 4, 1, 3, 2)
    ).reshape(NCORES, 128, BPC)

    nh = np.ascontiguousarray(-h_t.reshape(4, 128).T).astype(np.float32)
    im = np.tile(np.arange(128, dtype=np.float16), (128, 1))
    inn = np.tile(np.arange(NV, dtype=np.float16), (128, 1))
    oh = np.zeros((128, 64), np.float16)
    for g in range(8):
        oh[:, g * 8 + g] = 1.0
    idm = np.eye(8, dtype=np.float32)

    in_maps = []
    for k in range(NCORES):
        in_maps.append({
            "xt": xt8[k], "am": am8[k], "bn": bn8[k],
            "nh": nh, "im": im, "inn": inn, "oh": oh, "idm": idm,
        })
    return in_maps


def kernel(h_t, cache_h, word_ids):
    from concourse.bass_utils import run_bass_kernel_spmd

    if "nc" not in _CACHE:
        _CACHE["nc"] = _build_program()
    nc = _CACHE["nc"]

    in_maps = _prep_inputs(h_t, cache_h, word_ids)
    res = run_bass_kernel_spmd(nc, in_maps, list(range(NCORES)))

    hist = np.zeros((128, NV), np.float64)
    for k in range(NCORES):
        hist += res.results[k]["hist"].astype(np.float64)
    cache_p = hist.T.ravel()[:VOCAB]

    m = cache_p.max()
    lse = m + np.log(np.exp(cache_p - m).sum())
    out = (cache_p - lse).astype(np.float32)
    return out[None, :]


# revision 3
# speedup vs baseline: 1.0059x; 1.0059x over previous
"""Trainium2 Bass kernel: cache-distance -> exp kernel -> vocab histogram -> log_softmax.

Math (per cache row i): kern_i = exp(||cache_h[i] - h_t|| / 0.2)
                        cache_p[v] = sum_{i: word_ids[i]==v} kern_i
                        out = log_softmax(cache_p)[None, :]

Device strategy (8 cores, cache sharded along N):
  - cache slice uploaded pre-transposed [D=512, 32768] so D lives on SBUF partitions
  - ACT computes (x - h)^2 via Square activation with per-partition bias -h
  - PE reduces over D with one-hot-column [128,8] lhsT matmuls accumulating
    squared distances for 8 row-groups into one PSUM tile [8, 512]
  - ACT: kern = exp(exp(0.5*ln(25*d2)))  (Ln+Exp share one table set; avoids Sqrt)
  - PE transposes kern into [128, batch] orientation
  - histogram via outer-product matmul: hist[m,n] += sum_p kern_p *
      (m == wid_p % 128) * (n == wid_p // 128), with one-hots built by DVE
      fused tensor_scalar (is_equal, mult) against iota constants; PSUM
      accumulates all 256 batches into a single [128, 394] f32 tile.
Host: sum the 8 partial histograms, reorder to [V], log_softmax (tiny).
"""

import os
import sys

for _p in ("/root/.axon_site", "/root/.axon_site/_ro/trn_rl_repo",
           "/root/.axon_site/_ro/pypackages"):
    if os.path.isdir(_p) and _p not in sys.path:
        sys.path.append(_p)

import numpy as np

VOCAB = 50257
N_CACHE = 262144
D = 512
SMOOTH = 0.2
NCORES = 8
RPC = N_CACHE // NCORES        # 32768 rows per core
NCHUNK = 8
CHUNK = RPC // NCHUNK          # 4096 rows per chunk
GPC = CHUNK // 512             # 8 groups of 512 rows per chunk
NV = 394                       # hist free dim: wid // 128 in [0, 393), padded even
BPC = RPC // 128               # 256 batches of 128 elements per core

_CACHE = {}


def _build_program():
    import concourse.bacc as bacc
    import concourse.tile as tile
    import concourse.mybir as mybir

    f32, f16 = mybir.dt.float32, mybir.dt.float16
    AF = mybir.ActivationFunctionType
    ALU = mybir.AluOpType

    nc = bacc.Bacc("TRN2", target_bir_lowering=False, debug=False,
                   num_devices=NCORES)

    xt_d = nc.dram_tensor("xt", [D, RPC], f32, kind="ExternalInput")
    am_d = nc.dram_tensor("am", [128, BPC], f32, kind="ExternalInput")
    bn_d = nc.dram_tensor("bn", [128, BPC], f32, kind="ExternalInput")
    nh_d = nc.dram_tensor("nh", [128, 4], f32, kind="ExternalInput")
    im_d = nc.dram_tensor("im", [128, 128], f16, kind="ExternalInput")
    in_d = nc.dram_tensor("inn", [128, NV], f16, kind="ExternalInput")
    oh_d = nc.dram_tensor("oh", [128, 64], f16, kind="ExternalInput")
    id_d = nc.dram_tensor("idm", [8, 8], f32, kind="ExternalInput")
    hist_d = nc.dram_tensor("hist", [128, NV], f32, kind="ExternalOutput")

    with tile.TileContext(nc) as tc:
        with (
            tc.tile_pool(name="const", bufs=1) as cpool,
            tc.tile_pool(name="x", bufs=4) as xpool,
            tc.tile_pool(name="sq", bufs=4) as sqpool,
            tc.tile_pool(name="s", bufs=4) as spool,
            tc.tile_pool(name="kt", bufs=2) as ktpool,
            tc.tile_pool(name="a", bufs=6) as apool,
            tc.tile_pool(name="b", bufs=6) as bpool,
            tc.tile_pool(name="out", bufs=1) as opool,
            tc.tile_pool(name="pshist", bufs=1, space="PSUM") as pshist,
            tc.tile_pool(name="psdist", bufs=3, space="PSUM") as psdist,
            tc.tile_pool(name="pskt", bufs=2, space="PSUM") as pskt,
        ):
            am = cpool.tile([128, BPC], f32)
            nc.sync.dma_start(am[:], am_d.ap())
            bn = cpool.tile([128, BPC], f32)
            nc.sync.dma_start(bn[:], bn_d.ap())
            nh = cpool.tile([128, 4], f32)
            nc.sync.dma_start(nh[:], nh_d.ap())
            im = cpool.tile([128, 128], f16)
            nc.sync.dma_start(im[:], im_d.ap())
            inn = cpool.tile([128, NV], f16)
            nc.sync.dma_start(inn[:], in_d.ap())
            oh = cpool.tile([128, 64], f16)
            nc.sync.dma_start(oh[:], oh_d.ap())
            idm = cpool.tile([8, 8], f32)
            nc.sync.dma_start(idm[:], id_d.ap())

            hist = pshist.tile([128, NV], f32)
            xt_ap = xt_d.ap()

            PAIR = 2  # chunks per Ln/Exp phase (halves ACT table-set reloads)
            for pr in range(NCHUNK // PAIR):
                dists = []
                for ch in range(pr * PAIR, (pr + 1) * PAIR):
                    dist = psdist.tile([8, 512], f32)
                    mm = 0
                    for c in range(4):
                        x = xpool.tile([128, CHUNK], f32)
                        nc.sync.dma_start(
                            x[:],
                            xt_ap[c * 128:(c + 1) * 128,
                                  ch * CHUNK:(ch + 1) * CHUNK],
                        )
                        sq = sqpool.tile([128, CHUNK], f16)
                        nc.scalar.activation(sq[:], x[:], AF.Square,
                                             bias=nh[:, c:c + 1])
                        for g in range(GPC):
                            nc.tensor.matmul(
                                dist[:],
                                oh[:, g * 8:(g + 1) * 8],
                                sq[:, g * 512:(g + 1) * 512],
                                start=(mm == 0),
                                stop=(mm == 4 * GPC - 1),
                            )
                            mm += 1
                    dists.append((ch, dist))

                kerns = []
                for ch, dist in dists:
                    lg = spool.tile([8, 512], f32)
                    nc.scalar.activation(lg[:], dist[:], AF.Ln, scale=25.0)
                    d5 = spool.tile([8, 512], f32)
                    nc.scalar.activation(d5[:], lg[:], AF.Exp, scale=0.5)
                    kern = spool.tile([8, 512], f32)
                    nc.scalar.activation(kern[:], d5[:], AF.Exp)
                    kerns.append((ch, kern))

                for ch, kern in kerns:
                    ktp = pskt.tile([128, 32], f32)
                    for c4 in range(4):
                        nc.tensor.transpose(
                            ktp[:, c4 * 8:(c4 + 1) * 8],
                            kern[:, c4 * 128:(c4 + 1) * 128],
                            idm[:],
                        )
                    kt = ktpool.tile([128, 32], f32)
                    nc.scalar.copy(kt[:], ktp[:])

                    for c4 in range(4):
                        for g in range(GPC):
                            bl = c4 * 8 + g
                            b = ch * 32 + bl
                            A = apool.tile([128, 128], f16)
                            nc.vector.tensor_scalar(
                                A[:], im[:], am[:, b:b + 1], kt[:, bl:bl + 1],
                                ALU.is_equal, ALU.mult,
                            )
                            B = bpool.tile([128, NV], f16)
                            nc.vector.tensor_scalar(
                                B[:], inn[:], bn[:, b:b + 1], None,
                                ALU.is_equal,
                            )
                            nc.tensor.matmul(
                                hist[:], A[:], B[:],
                                start=(b == 0), stop=(b == BPC - 1),
                            )

            hist_sb = opool.tile([128, NV], f32)
            nc.scalar.copy(hist_sb[:], hist[:])
            nc.sync.dma_start(hist_d.ap(), hist_sb[:])

    nc.compile()
    return nc


def _prep_inputs(h_t, cache_h, word_ids):
    h_t = np.asarray(h_t, dtype=np.float32)
    cache_h = np.ascontiguousarray(np.asarray(cache_h, dtype=np.float32))
    word_ids = np.asarray(word_ids, dtype=np.int64)

    # [core, D, RPC] transposed cache slices
    xt8 = np.ascontiguousarray(
        cache_h.reshape(NCORES, RPC, D).transpose(0, 2, 1))

    w8 = word_ids.reshape(NCORES, NCHUNK, GPC, 4, 128)  # [core, ch, g, c, p]
    am8 = np.ascontiguousarray(
        (w8 % 128).astype(np.float32).transpose(0, 4, 1, 3, 2)
    ).reshape(NCORES, 128, BPC)
    bn8 = np.ascontiguousarray(
        (w8 // 128).astype(np.float32).transpose(0, 4, 1, 3, 2)
    ).reshape(NCORES, 128, BPC)

    nh = np.ascontiguousarray(-h_t.reshape(4, 128).T).astype(np.float32)
    im = np.tile(np.arange(128, dtype=np.float16), (128, 1))
    inn = np.tile(np.arange(NV, dtype=np.float16), (128, 1))
    oh = np.zeros((128, 64), np.float16)
    for g in range(8):
        oh[:, g * 8 + g] = 1.0
    idm = np.eye(8, dtype=np.float32)

    in_maps = []
    for k in range(NCORES):
        in_maps.append({
            "xt": xt8[k], "am": am8[k], "bn": bn8[k],
            "nh": nh, "im": im, "inn": inn, "oh": oh, "idm": idm,
        })
    return in_maps


def kernel(h_t, cache_h, word_ids):
    from concourse.bass_utils import run_bass_kernel_spmd

    if "nc" not in _CACHE:
        _CACHE["nc"] = _build_program()
    nc = _CACHE["nc"]

    in_maps = _prep_inputs(h_t, cache_h, word_ids)
    res = run_bass_kernel_spmd(nc, in_maps, list(range(NCORES)))

    hist = np.zeros((128, NV), np.float64)
    for k in range(NCORES):
        hist += res.results[k]["hist"].astype(np.float64)
    cache_p = hist.T.ravel()[:VOCAB]

    m = cache_p.max()
    lse = m + np.log(np.exp(cache_p - m).sum())
    out = (cache_p - lse).astype(np.float32)
    return out[None, :]


# revision 5
# speedup vs baseline: 1.2187x; 1.2115x over previous
"""Trainium2 Bass kernel: cache-distance -> exp kernel -> vocab histogram -> log_softmax.

Math (per cache row i): kern_i = exp(||cache_h[i] - h_t|| / 0.2)
                        cache_p[v] = sum_{i: word_ids[i]==v} kern_i
                        out = log_softmax(cache_p)[None, :]

Device strategy (8 cores, cache sharded along N):
  - cache slice uploaded pre-transposed [D=512, 32768] so D lives on SBUF partitions
  - ACT computes (x - h)^2 via Square activation with per-partition bias -h
  - PE reduces over D with one-hot-column [128,8] lhsT matmuls accumulating
    squared distances for 8 row-groups into one PSUM tile [8, 512]
  - ACT: kern = exp(exp(0.5*ln(25*d2)))  (Ln+Exp share one table set; avoids Sqrt)
  - PE transposes kern into [128, batch] orientation
  - histogram via outer-product matmul: hist[m,n] += sum_p kern_p *
      (m == wid_p % 128) * (n == wid_p // 128), with one-hots built by DVE
      fused tensor_scalar (is_equal, mult) against iota constants; PSUM
      accumulates all 256 batches into a single [128, 394] f32 tile.
Host: sum the 8 partial histograms, reorder to [V], log_softmax (tiny).
"""

import os
import sys

for _p in ("/root/.axon_site", "/root/.axon_site/_ro/trn_rl_repo",
           "/root/.axon_site/_ro/pypackages"):
    if os.path.isdir(_p) and _p not in sys.path:
        sys.path.append(_p)

import numpy as np

VOCAB = 50257
N_CACHE = 262144
D = 512
SMOOTH = 0.2
NCORES = 8
RPC = N_CACHE // NCORES        # 32768 rows per core
NCHUNK = 8
CHUNK = RPC // NCHUNK          # 4096 rows per chunk
GPC = CHUNK // 512             # 8 groups of 512 rows per chunk
NV = 394                       # hist free dim: wid // 128 in [0, 393), padded even
BPC = RPC // 128               # 256 batches of 128 elements per core

_CACHE = {}


def _build_program():
    import concourse.bacc as bacc
    import concourse.tile as tile
    import concourse.mybir as mybir

    f32, f16 = mybir.dt.float32, mybir.dt.float16
    AF = mybir.ActivationFunctionType
    ALU = mybir.AluOpType

    nc = bacc.Bacc("TRN2", target_bir_lowering=False, debug=False,
                   num_devices=NCORES)

    xt_d = nc.dram_tensor("xt", [D, RPC], f32, kind="ExternalInput")
    am_d = nc.dram_tensor("am", [128, BPC], f32, kind="ExternalInput")
    bn_d = nc.dram_tensor("bn", [128, BPC], f32, kind="ExternalInput")
    nh_d = nc.dram_tensor("nh", [128, 4], f32, kind="ExternalInput")
    im_d = nc.dram_tensor("im", [128, 128], f16, kind="ExternalInput")
    in_d = nc.dram_tensor("inn", [128, NV], f16, kind="ExternalInput")
    oh_d = nc.dram_tensor("oh", [128, 64], f16, kind="ExternalInput")
    id_d = nc.dram_tensor("idm", [8, 8], f32, kind="ExternalInput")
    hist_d = nc.dram_tensor("hist", [128, NV], f32, kind="ExternalOutput")

    with tile.TileContext(nc) as tc:
        with (
            tc.tile_pool(name="const", bufs=1) as cpool,
            tc.tile_pool(name="x", bufs=6) as xpool,
            tc.tile_pool(name="sq", bufs=4) as sqpool,
            tc.tile_pool(name="s", bufs=6) as spool,
            tc.tile_pool(name="kt", bufs=3) as ktpool,
            tc.tile_pool(name="a", bufs=34) as apool,
            tc.tile_pool(name="b", bufs=34) as bpool,
            tc.tile_pool(name="out", bufs=1) as opool,
            tc.tile_pool(name="pshist", bufs=1, space="PSUM") as pshist,
            tc.tile_pool(name="psdist", bufs=4, space="PSUM") as psdist,
            tc.tile_pool(name="pskt", bufs=2, space="PSUM") as pskt,
        ):
            am = cpool.tile([128, BPC], f32)
            nc.sync.dma_start(am[:], am_d.ap())
            bn = cpool.tile([128, BPC], f32)
            nc.sync.dma_start(bn[:], bn_d.ap())
            nh = cpool.tile([128, 4], f32)
            nc.sync.dma_start(nh[:], nh_d.ap())
            im = cpool.tile([128, 128], f16)
            nc.sync.dma_start(im[:], im_d.ap())
            inn = cpool.tile([128, NV], f16)
            nc.sync.dma_start(inn[:], in_d.ap())
            oh = cpool.tile([128, 64], f16)
            nc.sync.dma_start(oh[:], oh_d.ap())
            idm = cpool.tile([8, 8], f32)
            nc.sync.dma_start(idm[:], id_d.ap())

            hist = pshist.tile([128, NV], f32)
            xt_ap = xt_d.ap()

            # Software-pipelined emission (pairs of chunks). For pair p the
            # kern/transpose/one-hot/hist work is emitted during pair p+1's
            # load+dist phase so the PE stream ([transp][dist][hist]) never
            # waits on the serial ACT chain and HAM stays warm.
            PAIR = 2

            def emit_dist(ch):
                dist = psdist.tile([8, 512], f32)
                mm = 0
                for c in range(4):
                    x = xpool.tile([128, CHUNK], f32)
                    nc.sync.dma_start(
                        x[:],
                        xt_ap[c * 128:(c + 1) * 128,
                              ch * CHUNK:(ch + 1) * CHUNK],
                    )
                    sq = sqpool.tile([128, CHUNK], f16)
                    nc.scalar.activation(sq[:], x[:], AF.Square,
                                         bias=nh[:, c:c + 1])
                    for g in range(GPC):
                        nc.tensor.matmul(
                            dist[:],
                            oh[:, g * 8:(g + 1) * 8],
                            sq[:, g * 512:(g + 1) * 512],
                            start=(mm == 0),
                            stop=(mm == 4 * GPC - 1),
                        )
                        mm += 1
                return dist

            def emit_kern(dist):
                lg = spool.tile([8, 512], f32)
                nc.scalar.activation(lg[:], dist[:], AF.Ln, scale=25.0)
                d5 = spool.tile([8, 512], f32)
                nc.scalar.activation(d5[:], lg[:], AF.Exp, scale=0.5)
                kern = spool.tile([8, 512], f32)
                nc.scalar.activation(kern[:], d5[:], AF.Exp)
                return kern

            def emit_transp(kern):
                ktp = pskt.tile([128, 32], f32)
                for c4 in range(4):
                    nc.tensor.transpose(
                        ktp[:, c4 * 8:(c4 + 1) * 8],
                        kern[:, c4 * 128:(c4 + 1) * 128],
                        idm[:],
                    )
                kt = ktpool.tile([128, 32], f32)
                nc.scalar.copy(kt[:], ktp[:])
                return kt

            def emit_ab(ch, kt):
                abs_ = []
                for bl in range(32):
                    b = ch * 32 + bl
                    A = apool.tile([128, 128], f16)
                    nc.vector.tensor_scalar(
                        A[:], im[:], am[:, b:b + 1], kt[:, bl:bl + 1],
                        ALU.is_equal, ALU.mult,
                    )
                    B = bpool.tile([128, NV], f16)
                    nc.vector.tensor_scalar(
                        B[:], inn[:], bn[:, b:b + 1], None, ALU.is_equal,
                    )
                    abs_.append((b, A, B))
                return abs_

            def emit_hist(abs_):
                for b, A, B in abs_:
                    nc.tensor.matmul(
                        hist[:], A[:], B[:],
                        start=(b == 0), stop=(b == BPC - 1),
                    )

            NPAIR = NCHUNK // PAIR
            prev = None  # [(ch, dist), ...] of previous pair
            for pr in range(NPAIR):
                ab_waves = []
                if prev is not None:
                    kerns = [emit_kern(dist) for _, dist in prev]
                    kts = [emit_transp(k) for k in kerns]
                    ab_waves = [emit_ab(ch, kt)
                                for (ch, _), kt in zip(prev, kts)]
                cur = []
                for ch in range(pr * PAIR, (pr + 1) * PAIR):
                    cur.append((ch, emit_dist(ch)))
                for abs_ in ab_waves:
                    emit_hist(abs_)
                prev = cur
            # epilogue: drain last pair
            kerns = [emit_kern(dist) for _, dist in prev]
            kts = [emit_transp(k) for k in kerns]
            for (ch, _), kt in zip(prev, kts):
                emit_hist(emit_ab(ch, kt))

            hist_sb = opool.tile([128, NV], f32)
            nc.scalar.copy(hist_sb[:], hist[:])
            nc.sync.dma_start(hist_d.ap(), hist_sb[:])

    nc.compile()
    return nc


def _prep_inputs(h_t, cache_h, word_ids):
    h_t = np.asarray(h_t, dtype=np.float32)
    cache_h = np.ascontiguousarray(np.asarray(cache_h, dtype=np.float32))
    word_ids = np.asarray(word_ids, dtype=np.int64)

    # [core, D, RPC] transposed cache slices
    xt8 = np.ascontiguousarray(
        cache_h.reshape(NCORES, RPC, D).transpose(0, 2, 1))

    w8 = word_ids.reshape(NCORES, NCHUNK, GPC, 4, 128)  # [core, ch, g, c, p]
    am8 = np.ascontiguousarray(
        (w8 % 128).astype(np.float32).transpose(0, 4, 1, 3, 2)
    ).reshape(NCORES, 128, BPC)
    bn8 = np.ascontiguousarray(
        (w8 // 128).astype(np.float32).transpose(0, 4, 1, 3, 2)
    ).reshape(NCORES, 128, BPC)

    nh = np.ascontiguousarray(-h_t.reshape(4, 128).T).astype(np.float32)
    im = np.tile(np.arange(128, dtype=np.float16), (128, 1))
    inn = np.tile(np.arange(NV, dtype=np.float16), (128, 1))
    oh = np.zeros((128, 64), np.float16)
    for g in range(8):
        oh[:, g * 8 + g] = 1.0
    idm = np.eye(8, dtype=np.float32)

    in_maps = []
    for k in range(NCORES):
        in_maps.append({
            "xt": xt8[k], "am": am8[k], "bn": bn8[k],
            "nh": nh, "im": im, "inn": inn, "oh": oh, "idm": idm,
        })
    return in_maps


def kernel(h_t, cache_h, word_ids):
    from concourse.bass_utils import run_bass_kernel_spmd

    if "nc" not in _CACHE:
        _CACHE["nc"] = _build_program()
    nc = _CACHE["nc"]

    in_maps = _prep_inputs(h_t, cache_h, word_ids)
    res = run_bass_kernel_spmd(nc, in_maps, list(range(NCORES)))

    hist = np.zeros((128, NV), np.float64)
    for k in range(NCORES):
        hist += res.results[k]["hist"].astype(np.float64)
    cache_p = hist.T.ravel()[:VOCAB]

    m = cache_p.max()
    lse = m + np.log(np.exp(cache_p - m).sum())
    out = (cache_p - lse).astype(np.float32)
    return out[None, :]
